# revision 1
# baseline (speedup 1.0000x reference)
"""Trainium2 Bass kernel for nn_CategorySpecificInitNet (moe_routing).

kernel(**inputs) takes the FULL unsharded inputs (keys as in
reference.setup_inputs()) and returns the FULL [B, 128] float32 output.

Strategy — expert-parallel, per the spec sharding hint's dispatch-by-category
alternative:
  - the host sharding layer dispatches rows to cores by category (the
    "all-to-all dispatch by category" of expert-parallel, realized where
    all sharding happens in this harness): rows are stably sorted by
    cat_idx and core k receives category k's rows, zero-padded to a
    static per-core capacity (max category count rounded up to the
    512-row tile size);
  - every core runs the shared encoder plus exactly ONE decoder (its
    category's), so no routing, masking, or gather happens per row —
    the decoder FLOPs drop 8x vs computing all decoders densely;
  - the encoder's linear third layer is constant-folded into the
    decoder's first layer on the host (W_f = We3 @ Wd1_k,
    b_f = Wd1_k^T be3 + bd1_k — exact algebra, ~0.1% of the FLOPs),
    removing one full matmul stage from the device;
  - all compute is feature-major [features(partitions), rows(free)], so
    no transposes are ever needed on device (the host passes features
    pre-transposed); outputs come back [128, cap] and the host
    inverse-permutes rows during unsharding.
  - per-core row tiles of 512; the decoder stages are software-pipelined
    one tile behind the encoder so the PE never waits on ACT/DVE
    relu latency.

Matmuls run in float32r (fp32 storage, full PE rate at N=512, ~tf32-grade
multiply precision on HW — measured ~3e-4 max rel error vs the fp32
reference, 17x better than bf16 at the same PE throughput).
"""
import sys

for _p in ("/opt/trn_rl_repo",):
    if _p not in sys.path:
        sys.path.append(_p)

import numpy as np

import concourse.bass as bass
import concourse.bacc as bacc
import concourse.mybir as mybir
import concourse.tile as tile
from concourse import bass_utils

FR = mybir.dt.float32r
F32 = mybir.dt.float32
Alu = mybir.AluOpType
ActF = mybir.ActivationFunctionType

B, C, H1, H2, HO = 32768, 768, 512, 256, 256
DH, LAT, K = 256, 128, 8
N_CORES = 8
TILE = 512
# bias_all columns: be1[4] be2[2] bf[2](=Wd1^T be3 + bd1) bd2[2] bd3[1]
OB1, OB2, OD1, OD2, OD3 = 0, 4, 6, 8, 10
NBIAS = 11


def _build_nc(cap, tile_n=512, ps_w_bufs=6, dp_bufs=2, ps_o_bufs=2, split=(3, 3), tail_pos=1, ap_bufs=3, fp_bufs=2):
    assert cap % 256 == 0
    tiles = [tile_n] * (cap // tile_n)
    if cap % tile_n:
        tiles.insert(tail_pos if tail_pos is not None else len(tiles),
                     cap % tile_n)
    offs = [sum(tiles[:i]) for i in range(len(tiles))]
    nt = len(tiles)
    nc = bacc.Bacc(name="catnet_ep")

    fT = nc.dram_tensor("fT", (C, cap), FR, kind="ExternalInput")
    we1 = nc.dram_tensor("we1", (C, H1), FR, kind="ExternalInput")
    we2 = nc.dram_tensor("we2", (H1, H2), FR, kind="ExternalInput")
    wd1 = nc.dram_tensor("wd1", (H2, DH), FR, kind="ExternalInput")  # We3 @ Wd1
    wd2 = nc.dram_tensor("wd2", (DH, DH), FR, kind="ExternalInput")
    wd3 = nc.dram_tensor("wd3", (DH, LAT), FR, kind="ExternalInput")
    bias_all = nc.dram_tensor("bias_all", (128, NBIAS), F32, kind="ExternalInput")
    out = nc.dram_tensor("out", (LAT, cap), F32, kind="ExternalOutput")

    nC, nH1, nH2, nHO, nDH = C // 128, H1 // 128, H2 // 128, HO // 128, DH // 128

    with tile.TileContext(nc) as tc:
        with (
            tc.tile_pool(name="wp", bufs=1) as wp,
            tc.tile_pool(name="fp", bufs=fp_bufs) as fp,
            tc.tile_pool(name="ap", bufs=ap_bufs) as ap,
            tc.tile_pool(name="dp", bufs=dp_bufs) as dp,
            tc.tile_pool(name="ps_w", bufs=ps_w_bufs, space="PSUM") as ps_w,
            tc.tile_pool(name="ps_o", bufs=ps_o_bufs, space="PSUM") as ps_o,
        ):
            # we1 first, in two half-tensor DMAs: per-HWDGE-DMA queue issue
            # costs ~0.6us, so 2 DMAs beats 6 for total latency while still
            # letting the first L1 matmuls start after the first half lands
            we1_t = wp.tile([128, nC, H1], FR, tag="we1")
            we1_r = we1.rearrange("(c p) h -> p c h", p=128)
            c0 = 0
            for w in split:
                nc.gpsimd.dma_start(we1_t[:, c0:c0 + w, :], we1_r[:, c0:c0 + w, :])
                c0 += w
            bias_t = wp.tile([128, NBIAS], F32, tag="bias")
            nc.gpsimd.dma_start(bias_t[:], bias_all[:])
            we2_t = wp.tile([128, nH1, H2], FR, tag="we2")
            nc.gpsimd.dma_start(we2_t[:], we2.rearrange("(c p) h -> p c h", p=128))
            wd1_t = wp.tile([128, nH2, DH], FR, tag="wd1")
            nc.gpsimd.dma_start(wd1_t[:], wd1.rearrange("(c p) d -> p c d", p=128))
            wd2_t = wp.tile([128, nDH, DH], FR, tag="wd2")
            nc.gpsimd.dma_start(wd2_t[:], wd2.rearrange("(c p) d -> p c d", p=128))
            wd3_t = wp.tile([128, nDH, LAT], FR, tag="wd3")
            nc.gpsimd.dma_start(wd3_t[:], wd3.rearrange("(c p) d -> p c d", p=128))

            def emit_enc(t):
                tn = tiles[t]
                sl = slice(offs[t], offs[t] + tn)
                ftb_fl = fp.tile([128, nC, tile_n], FR, tag="ft")
                ftb = ftb_fl[:, :, :tn]
                if t == 0:
                    fr = fT.rearrange("(c p) b -> p c b", p=128)[:, :, sl]
                    c0 = 0
                    for w in split:
                        nc.sync.dma_start(ftb[:, c0:c0 + w, :], fr[:, c0:c0 + w, :])
                        c0 += w
                else:
                    # alternate queues so consecutive feature tiles stream in
                    # parallel instead of serializing on one HWDGE queue
                    eng = nc.sync if t % 2 == 0 else nc.gpsimd
                    eng.dma_start(
                        ftb[:], fT.rearrange("(c p) b -> p c b", p=128)[:, :, sl])
                pwsl = slice(0, tn)
                a1 = []
                for m in range(nH1):
                    pw_fl = ps_w.tile([128, tile_n], F32, tag="pw")
                    pw = pw_fl[:, :tn]
                    for c in range(nC):
                        nc.tensor.matmul(pw[:], we1_t[:, c, bass.ts(m, 128)],
                                         ftb[:, c, :],
                                         start=(c == 0), stop=(c == nC - 1))
                    x_fl = ap.tile([128, tile_n], FR, tag=f"a1_{m}")
                    x = x_fl[:, :tn]
                    nc.scalar.activation(x[:], pw[:], ActF.Relu,
                                         bias=bias_t[:, OB1 + m:OB1 + m + 1])
                    a1.append(x)
                a2 = []
                for m in range(nH2):
                    pw_fl = ps_w.tile([128, tile_n], F32, tag="pw")
                    pw = pw_fl[:, :tn]
                    for c in range(nH1):
                        nc.tensor.matmul(pw[:], we2_t[:, c, bass.ts(m, 128)], a1[c][:],
                                         start=(c == 0), stop=(c == nH1 - 1))
                    x_fl = ap.tile([128, tile_n], FR, tag=f"a2_{m}")
                    x = x_fl[:, :tn]
                    if m % 2 == 0:
                        nc.vector.tensor_scalar(x[:], pw[:],
                                                bias_t[:, OB2 + m:OB2 + m + 1],
                                                0.0, Alu.add, Alu.max)
                    else:
                        nc.scalar.activation(x[:], pw[:], ActF.Relu,
                                             bias=bias_t[:, OB2 + m:OB2 + m + 1])
                    a2.append(x)
                return a2

            def emit_d1(t, h):
                tn = tiles[t]
                d1 = []
                for m in range(nDH):
                    pw_fl = ps_w.tile([128, tile_n], F32, tag="pw")
                    pw = pw_fl[:, :tn]
                    for c in range(nH2):
                        nc.tensor.matmul(pw[:], wd1_t[:, c, bass.ts(m, 128)], h[c][:],
                                         start=(c == 0), stop=(c == nH2 - 1))
                    x_fl = dp.tile([128, tile_n], FR, tag=f"d1_{m}")
                    x = x_fl[:, :tn]
                    if m % 2 == 1:
                        nc.vector.tensor_scalar(x[:], pw[:],
                                                bias_t[:, OD1 + m:OD1 + m + 1],
                                                0.0, Alu.add, Alu.max)
                    else:
                        nc.scalar.activation(x[:], pw[:], ActF.Relu,
                                             bias=bias_t[:, OD1 + m:OD1 + m + 1])
                    d1.append(x)
                return d1

            def emit_d2_d3_store(t, d1):
                tn = tiles[t]
                d2 = []
                for m in range(nDH):
                    pw_fl = ps_w.tile([128, tile_n], F32, tag="pw")
                    pw = pw_fl[:, :tn]
                    for c in range(nDH):
                        nc.tensor.matmul(pw[:], wd2_t[:, c, bass.ts(m, 128)], d1[c][:],
                                         start=(c == 0), stop=(c == nDH - 1))
                    x_fl = dp.tile([128, tile_n], FR, tag=f"d2_{m}")
                    x = x_fl[:, :tn]
                    bb = bias_t[:, OD2 + m:OD2 + m + 1]
                    if m % 2 == 0:
                        nc.vector.tensor_scalar(x[:], pw[:], bb, 0.0, Alu.add, Alu.max)
                    else:
                        nc.scalar.activation(x[:], pw[:], ActF.Relu, bias=bb)
                    d2.append(x)
                po_fl = ps_o.tile([128, tile_n], F32, tag="out")
                po = po_fl[:, :tn]
                for c in range(nDH):
                    nc.tensor.matmul(po[:], wd3_t[:, c, :], d2[c][:],
                                     start=(c == 0), stop=(c == nDH - 1))
                osb_fl = ap.tile([128, tile_n], F32, tag="osb")
                osb = osb_fl[:, :tn]
                nc.scalar.activation(osb[:], po[:], ActF.Identity,
                                     bias=bias_t[:, OD3:OD3 + 1])
                nc.gpsimd.dma_start(out[:, offs[t]:offs[t] + tn], osb[:])

            # decoder runs one tile behind the encoder: PE order per step is
            # [enc t][d2/d3 t-1][d1 t], hiding ACT/DVE relu latency behind
            # independent matmuls
            pend = None
            for t in range(nt):
                h = emit_enc(t)
                if pend is not None:
                    emit_d2_d3_store(pend[0], pend[1])
                d1 = emit_d1(t, h)
                pend = (t, d1)
            emit_d2_d3_store(pend[0], pend[1])

    nc.finalize()
    return nc


def _pack_inputs(features, We1, be1, We2, be2, We3, be3,
                 Wd1, bd1, Wd2, bd2, Wd3, bd3, cat_idx, cap):
    """Dispatch rows to cores by category (expert-parallel sharding)."""
    features = np.asarray(features, np.float32)
    cat = np.asarray(cat_idx).astype(np.int64)
    order = np.argsort(cat, kind="stable")
    counts = np.bincount(cat, minlength=N_CORES)
    starts = np.zeros(N_CORES + 1, np.int64)
    np.cumsum(counts, out=starts[1:])

    def chunkcols(b):
        b = np.asarray(b, np.float32).reshape(-1)
        return b.reshape(-1, 128).T

    enc = dict(
        we1=np.asarray(We1, np.float32), we2=np.asarray(We2, np.float32),
    )
    We3f = np.asarray(We3, np.float32)
    be3f = np.asarray(be3, np.float32)
    maps, rows_per_core = [], []
    for k in range(N_CORES):
        rows = order[starts[k]:starts[k + 1]]
        rows_per_core.append(rows)
        f = np.zeros((cap, C), np.float32)
        f[:len(rows)] = features[rows]
        bias_all = np.zeros((128, NBIAS), np.float32)
        bias_all[:, OB1:OB1 + 4] = chunkcols(be1)
        bias_all[:, OB2:OB2 + 2] = chunkcols(be2)
        wd1k = np.asarray(Wd1, np.float32)[k]
        bias_all[:, OD1:OD1 + 2] = chunkcols(
            wd1k.T @ be3f + np.asarray(bd1, np.float32)[k])
        bias_all[:, OD2:OD2 + 2] = chunkcols(np.asarray(bd2, np.float32)[k])
        bias_all[:, OD3:OD3 + 1] = chunkcols(np.asarray(bd3, np.float32)[k])
        m = dict(enc)
        m["fT"] = np.ascontiguousarray(f.T)
        m["wd1"] = We3f @ wd1k  # encoder L3 folded into decoder layer 1
        m["wd2"] = np.asarray(Wd2, np.float32)[k]
        m["wd3"] = np.asarray(Wd3, np.float32)[k]
        m["bias_all"] = bias_all
        maps.append(m)
    return maps, rows_per_core


_NC_CACHE = {}


def _get_nc(cap=4352):
    if cap not in _NC_CACHE:
        _NC_CACHE[cap] = _build_nc(cap)
    return _NC_CACHE[cap]


def kernel(**inputs) -> np.ndarray:
    cat = np.asarray(inputs["cat_idx"]).astype(np.int64)
    counts = np.bincount(cat, minlength=N_CORES)
    cap = max(256, int(-(-counts.max() // 256) * 256))
    maps, rows_per_core = _pack_inputs(**inputs, cap=cap)
    nc = _get_nc(cap)
    res = bass_utils.run_bass_kernel_spmd(nc, maps, core_ids=list(range(N_CORES)))
    latent = np.zeros((B, LAT), np.float32)
    for k, r in enumerate(res.results):
        rows = rows_per_core[k]
        latent[rows] = r["out"][:, :len(rows)].T
    return latent

